# revision 1
# baseline (speedup 1.0000x reference)
"""Multi-head attention (B=2, L=2048, D=1024, H=16) on 8 trn2 cores.

Sharding: core c -> (batch b = c//4) x (head-group hg = c%4, 4 heads each).
W_q/W_k/W_v are column-split, W_o row-split; the 4 partial outputs per
batch are summed on the host (plus bo).

Per-core kernel (all matmuls lhsT.T @ rhs, contraction on partitions):
  inputs are host-transposed (xT = x.T, so d_model lands on partitions):
    QT[256,2048] = (Wq_s chunkT).T @ xqT   (accumulate over 8 d-chunks)
    KT, VT likewise.  V = PE-transpose(VT) per head -> V'[Lk, 65]
    (65th column = ones, used to compute the softmax denominator).
  attention per head, scores kept transposed (Lk on partitions):
    ST[128,512] = KT_h_chunk.T @ QT_h_bank          (K = dk = 64)
    P = Exp(ST * (1/sqrt(dk)) + mask_bias)          (one ACT op: scale+mask+exp)
    OT'[65,512] += V'_chunk.T @ P                   (row 64 = denominator)
    OT = OT'[0:64] * broadcast(1/OT'[64])           (DMA partition-broadcast)
  out[2048,1024] = (OT_all chunk).T @ Wo_s  (per-core partial, summed on host)
"""

import sys

for _p in ("/opt/trn_rl_repo",):
    if _p not in sys.path:
        sys.path.insert(0, _p)

import numpy as np

import concourse.bass as bass
import concourse.mybir as mybir
import concourse.tile as tile
from concourse import bacc
from concourse.bass import ts
from concourse.bass_utils import run_bass_kernel_spmd
from concourse.tile_rust import add_dep_helper

F32 = mybir.dt.float32

D_MODEL = 1024
NUM_HEADS = 16
D_K = 64
B = 2
L = 2048
N_CORES = 8
HPC = NUM_HEADS // 4  # heads per core (4)
SCALE = float(np.sqrt(D_K))
MASK_BIAS = -30000.0


def build_nc(L=L, D=D_MODEL, HPC=HPC, mm_dtype=mybir.dt.float32r):
    """Build the per-core Bass program (SPMD: same program, 8 cores).

    Wait-budget discipline (every DMA descriptor and every fp32r matmul
    carries a single hardware sync-wait; compute instructions may carry
    several because walrus splits them):
      * PE writes PSUM, ACT (ScalarE) evacuates PSUM, DVE stays off PSUM,
        so psum slot releases ride the ACT semaphore which the PE already
        tracks through its exp-output waits.
      * DMA-written SBUF slots use bufs=8 so a slot's previous writer sits
        on the same HW ring (WAW implied by ring FIFO); the one remaining
        wait is the readers' engine semaphore.  Explicit order chains keep
        the ring rotation deterministic.
      * softmax normalization broadcasts 1/denom with a K=1 matmul
        (ones.T @ recip_row) instead of a DRAM-bounce DMA.
      * zero "primer" matmuls open each accumulation group so real matmuls
        never join a group start with a data wait.
    """
    DK = D_K
    C = HPC * DK           # attention columns per core (256)
    CT = C // 128          # col tiles (2)
    DC = D // 128          # d_model chunks (8)
    LB = L // 512          # Lq banks (4)
    LT = L // 128          # Lk tiles (16)
    NH = max(1, L // 1024)  # x-chunk halves per d-chunk
    CW = L // NH            # x-chunk width

    MDT = mm_dtype  # dtype for every tensor feeding a matmul

    nc = bacc.Bacc("TRN2", target_bir_lowering=False, debug=False,
                   num_devices=N_CORES)

    xT = {n: nc.dram_tensor(f"x{n}T", [D, L], MDT, kind="ExternalInput").ap()
          for n in ("q", "k", "v")}
    w = {n: nc.dram_tensor(f"w{n}", [D, C], MDT, kind="ExternalInput").ap()
         for n in ("q", "k", "v")}
    wo = nc.dram_tensor("wo", [C, D], MDT, kind="ExternalInput").ap()
    bias = {n: nc.dram_tensor(f"b{n}", [C], F32, kind="ExternalInput").ap()
            for n in ("q", "k", "v")}
    mb = nc.dram_tensor("mb", [128, LT], F32, kind="ExternalInput").ap()
    id2 = nc.dram_tensor("id2", [128, DK], MDT, kind="ExternalInput").ap()
    onesd = nc.dram_tensor("ones", [1, DK], MDT, kind="ExternalInput").ap()
    ztd = nc.dram_tensor("ztc", [128, 128], MDT, kind="ExternalInput").ap()
    vod = nc.dram_tensor("vones", [128, LT * HPC], MDT,
                         kind="ExternalInput").ap()
    out = nc.dram_tensor("partial", [L, D], F32, kind="ExternalOutput").ap()

    Ident = mybir.ActivationFunctionType.Identity

    with tile.TileContext(nc) as tc:
        with (
            tc.tile_pool(name="consts", bufs=1) as consts,
            tc.tile_pool(name="persist", bufs=1) as persist,
            tc.tile_pool(name="xch", bufs=8) as xch,
            tc.tile_pool(name="work", bufs=3) as work,
            tc.tile_pool(name="ostg", bufs=8) as ostgp,
            tc.tile_pool(name="ps", bufs=8, space="PSUM") as psp,
        ):
            def prime(out_ps, rhs_ap, lhsT=None, start=True, stop=True):
                return nc.tensor.matmul(
                    out_ps, lhsT=(lhsT if lhsT is not None else rhs_ap[:, :1]),
                    rhs=rhs_ap, start=start, stop=stop,
                    skip_group_check=stop is True)

            def ps_tile(name=None, dtype=F32):
                return psp.tile([128, 512], dtype, tag="ps",
                                name=name or "pst")

            # ---- constants (SWDGE; x/out stay on the HWDGE rings) ----
            w_sb = {}
            for n in ("q", "k", "v"):
                w_sb[n] = consts.tile([128, DC, C], MDT, tag=f"w{n}",
                                      name=f"w{n}_sb")
                nc.gpsimd.dma_start(
                    out=w_sb[n], in_=w[n].rearrange("(c p) n -> p c n", p=128))
            wo_sb = consts.tile([128, CT, D], MDT, tag="wo")
            nc.gpsimd.dma_start(
                out=wo_sb, in_=wo.rearrange("(g p) n -> p g n", p=128))
            b_sb = {}
            for n in ("q", "k", "v"):
                b_sb[n] = consts.tile([128, CT], F32, tag=f"b{n}",
                                      name=f"b{n}_sb")
                nc.gpsimd.dma_start(
                    out=b_sb[n], in_=bias[n].rearrange("(t p) -> p t", p=128))
            mb_sb = consts.tile([128, LT], F32, tag="mb")
            nc.gpsimd.dma_start(out=mb_sb, in_=mb)
            id2_sb = consts.tile([128, DK], MDT, tag="id2")
            nc.gpsimd.dma_start(out=id2_sb, in_=id2)
            v_all = persist.tile([128, LT, HPC, DK + 1], MDT, tag="vall")
            nc.gpsimd.dma_start(
                out=v_all[:, :, :, DK],
                in_=vod.rearrange("p (c h) -> p c h", c=LT))
            ones_sb = consts.tile([1, DK], MDT, tag="ones")
            nc.gpsimd.dma_start(out=ones_sb, in_=onesd)
            zt = consts.tile([128, 128], MDT, tag="zt")
            nc.gpsimd.dma_start(out=zt, in_=ztd)

            # const primers: make PE observe each const-DMA semaphore once
            for l_, r_ in ((w_sb["q"][:, 0, 0:1], w_sb["q"][:, 0:2, :]),
                           (w_sb["k"][:, 0, 0:1], w_sb["k"][:, 0:2, :]),
                           (w_sb["v"][:, 0, 0:1], w_sb["v"][:, 0:2, :]),
                           (wo_sb[:, 0, 0:1], wo_sb[:, 0, 0:min(512, D)]),
                           (id2_sb[:, 0:1], id2_sb)):
                scr = ps_tile(name="scr")
                prime(scr[0:1, : r_.free_size()], r_, lhsT=l_)

            # ---- phase 1: projections -> QT/KT/VT [128, CT, L] ----
            projT = {}
            zz = [None]
            last_xdma = None
            for n in ("q", "k", "v"):
                dst = persist.tile([128, CT, L], MDT, tag=f"{n}t",
                                   name=f"{n}t_sb")
                projT[n] = dst
                ps_tiles = [ps_tile(name=f"ps_{n}_{i}")
                            for i in range(CT * LB)]
                prs = w_sb[n].rearrange("p c n -> p (c n)")
                if prs.free_size() < 512:   # small test configs only
                    if zz[0] is None:
                        zz[0] = consts.tile([128, 512], MDT, tag="zz",
                                            name="zz_sb")
                        nc.vector.memset(zz[0].bitcast(F32), 0.0)
                    prs = zz[0]
                for t_ in ps_tiles:
                    # the primer write must span the full bank so the psum
                    # pending-zero state stays uniform for the accumulates
                    prime(t_, prs[:, :512], lhsT=zt, start=True, stop=False)
                for hc in range(DC * NH):
                    dc, lh = hc // NH, hc % NH
                    xc = xch.tile([128, CW], MDT, tag="xc")
                    xd = nc.sync.dma_start(
                        out=xc, in_=xT[n][ts(dc, 128), ts(lh, CW)])
                    if last_xdma is not None:
                        # deterministic ring rotation (slot i <-> ring i)
                        add_dep_helper(xd.ins, last_xdma.ins, sync=False,
                                       reason="xdma-order")
                    last_xdma = xd
                    for ct in range(CT):
                        for lb2 in range(CW // 512):
                            lb = lh * (CW // 512) + lb2
                            nc.tensor.matmul(
                                ps_tiles[ct * LB + lb],
                                lhsT=w_sb[n][:, dc, ts(ct, 128)],
                                rhs=xc[:, ts(lb2, 512)],
                                start=False, stop=(dc == DC - 1))
                for ct in range(CT):
                    for lb in range(LB):
                        nc.scalar.activation(
                            dst[:, ct, ts(lb, 512)],
                            ps_tiles[ct * LB + lb], Ident,
                            bias=b_sb[n][:, ct:ct + 1])

            # ---- phase 1b: V' = [V | ones] in natural [Lk, 65] layout ----
            for h in range(HPC):
                g, po = h // 2, 64 * (h % 2)
                for c in range(LT):
                    pst = ps_tile(name="pst", dtype=MDT)
                    nc.tensor.transpose(
                        pst[:, :DK],
                        projT["v"][po:po + DK, g, ts(c, 128)],
                        id2_sb[po:po + DK, :])
                    nc.scalar.copy(
                        out=v_all[:, c, h, 0:DK], in_=pst[:, :DK])

            # ---- phase 2: attention ----
            ot_sb = persist.tile([128, CT, L], MDT, tag="ot")
            for h in range(HPC):
                g, po = h // 2, 64 * (h % 2)
                ot_ps = [ps_tile(name=f"ot_ps_{h}_{i}") for i in range(LB)]
                last_prime = None
                for t_ in ot_ps:
                    pp = prime(t_[:DK + 1, :], projT["q"][:, 0, 0:512],
                               lhsT=zt[:, :DK + 1], start=True, stop=False)
                    if last_prime is not None:
                        add_dep_helper(pp.ins, last_prime.ins, sync=False,
                                       reason="prime-order")
                    last_prime = pp
                for c in range(LT):
                    for lb in range(LB):
                        s_ps = ps_tile(name="s_ps")
                        smi = nc.tensor.matmul(
                            s_ps,
                            lhsT=projT["k"][po:po + DK, g, ts(c, 128)],
                            rhs=projT["q"][po:po + DK, g, ts(lb, 512)],
                            start=True, stop=True)
                        add_dep_helper(smi.ins, last_prime.ins, sync=False,
                                       reason="s-after-prime")
                        p_t = work.tile([128, 512], MDT, tag="p")
                        nc.scalar.activation(
                            p_t, s_ps, mybir.ActivationFunctionType.Exp,
                            bias=mb_sb[:, c:c + 1], scale=1.0 / SCALE)
                        nc.tensor.matmul(
                            ot_ps[lb][:DK + 1, :],
                            lhsT=v_all[:, c, h, :],
                            rhs=p_t,
                            start=False, stop=(c == LT - 1))
                for lb in range(LB):
                    ot_raw = work.tile([128, 512], F32, tag="otr")
                    nc.scalar.copy(out=ot_raw[:DK + 1, :],
                                   in_=ot_ps[lb][:DK + 1, :])
                    rc = work.tile([1, 512], MDT, tag="rc")
                    with nc.allow_low_precision(
                            reason="f32r-rounded reciprocal feeds the "
                                   "broadcast matmul; |denom|>=1"):
                        nc.vector.reciprocal(rc, ot_raw[DK:DK + 1, :])
                    ps_rep = ps_tile(name="ps_rep")
                    nc.tensor.matmul(ps_rep[:DK, :], lhsT=ones_sb, rhs=rc,
                                     start=True, stop=True)
                    rep_sb = work.tile([64, 512], F32, tag="rep")
                    nc.scalar.copy(out=rep_sb, in_=ps_rep[:DK, :])
                    head_dep = nc.vector.tensor_mul(
                        ot_sb[po:po + DK, g, ts(lb, 512)],
                        ot_raw[0:DK, :], rep_sb)

            # ---- phase 3: output projection (partial; summed on host) ----
            WOW = min(512, D)
            last_odma = None
            for t in range(LT):
                for half in range(D // WOW):
                    wps = ps_tile(name="wps")
                    for g in range(CT):
                        mmi = nc.tensor.matmul(
                            wps[:, :WOW],
                            lhsT=ot_sb[:, g, ts(t, 128)],
                            rhs=wo_sb[:, g, ts(half, WOW)],
                            start=(g == 0), stop=(g == CT - 1))
                    ostg = ostgp.tile([128, WOW], F32, tag="os")
                    nc.scalar.copy(out=ostg, in_=wps[:, :WOW])
                    od = nc.sync.dma_start(
                        out=out[ts(t, 128), ts(half, WOW)], in_=ostg)
                    if last_odma is not None:
                        add_dep_helper(od.ins, last_odma.ins, sync=False,
                                       reason="odma-order")
                    last_odma = od

    nc.compile()   # bacc lowering: event sems split multi-wait instructions
    return nc


def _strip_implied_dma_ring_waits(nc):
    """Drop DMA ring-semaphore waits that are implied by a compute-engine
    wait on the same descriptor.

    A recycled DMA-written SBUF slot gets two waits: the readers' engine
    semaphore (slot release) and the previous writer's DMA-ring semaphore
    (WAW).  The readers themselves data-waited on that previous DMA, so
    release >= WAW always; but DMA descriptors carry a single hardware
    sync-wait, so Tile's conservative pair fails walrus codegen.  Keep the
    engine wait, drop the ring wait.  Applied only to the x-chunk loads and
    output-staging stores, whose only DMA-semaphore deps are these WAW /
    WAR-release edges (their data comes from DRAM inputs or compute-engine
    writes, never from another DMA).
    """
    import concourse.mybir as _mb
    for ins in nc.inst_map.values():
        if type(ins).__name__ != "InstDMACopy":
            continue
        if not ins.outs:
            continue
        memref = getattr(ins.outs[0], "memref", "") or ""
        src_ref = getattr(ins.ins[0], "memref", "") if ins.ins else ""
        if not (memref.startswith("xc_") or (src_ref or "").startswith("ostg")):
            continue
        si = ins.sync_info
        if not si or not si.on_wait or len(si.on_wait) < 2:
            continue
        eng = [w for w in si.on_wait
               if not (w.ant_name or "").startswith(("DMAHW", "DMASW"))]
        if not eng:
            continue
        ins.sync_info = _mb.SyncInfo(on_wait=eng, on_update=list(si.on_update))


def make_in_maps(query, key, value, mask, Wq, bq, Wk, bk, Wv, bv, Wo, bo,
                 L=L, D=D_MODEL, HPC=HPC):
    """Host-side sharding: per-core input dicts."""
    DK = D_K
    C = HPC * DK
    LT = L // 128
    id2 = np.ascontiguousarray(
        np.tile(np.eye(DK, dtype=np.float32), (2, 1)))
    ones = np.ones((1, DK), np.float32)
    ztc = np.zeros((128, 128), np.float32)
    vones = np.ones((128, (L // 128) * HPC), np.float32)
    in_maps = []
    xTs = {}
    mbs = {}
    n_cores = (query.shape[0]) * (D // C)
    groups_per_batch = D // C
    for b in range(query.shape[0]):
        xTs[b] = {
            "q": np.ascontiguousarray(query[b].T),
            "k": np.ascontiguousarray(key[b].T),
            "v": np.ascontiguousarray(value[b].T),
        }
        mbf = np.where(mask[b, 0], np.float32(MASK_BIAS), np.float32(0.0))
        mbs[b] = np.ascontiguousarray(
            mbf.reshape(LT, 128).T.astype(np.float32))
    for c in range(n_cores):
        b, hg = divmod(c, groups_per_batch)
        sl = slice(hg * C, (hg + 1) * C)
        in_maps.append({
            "xqT": xTs[b]["q"], "xkT": xTs[b]["k"], "xvT": xTs[b]["v"],
            "wq": np.ascontiguousarray(Wq[:, sl]),
            "wk": np.ascontiguousarray(Wk[:, sl]),
            "wv": np.ascontiguousarray(Wv[:, sl]),
            "wo": np.ascontiguousarray(Wo[sl, :]),
            "bq": np.ascontiguousarray(bq[sl]),
            "bk": np.ascontiguousarray(bk[sl]),
            "bv": np.ascontiguousarray(bv[sl]),
            "mb": mbs[b],
            "id2": id2,
            "ones": ones, "ztc": ztc, "vones": vones,
        })
    return in_maps


_NC_CACHE = {}


def _get_nc(mm_dtype=mybir.dt.float32r):
    key = str(mm_dtype)
    if key not in _NC_CACHE:
        _NC_CACHE[key] = build_nc(mm_dtype=mm_dtype)
    return _NC_CACHE[key]


def run(inputs, mm_dtype=mybir.dt.float32r, trace=False):
    """Run on 8 cores; returns (full_output, BassKernelResults)."""
    inputs = {k: np.asarray(v) for k, v in inputs.items()}
    nc = _get_nc(mm_dtype)
    in_maps = make_in_maps(**inputs)
    res = run_bass_kernel_spmd(nc, in_maps, list(range(N_CORES)), trace=trace)
    groups_per_batch = N_CORES // B
    out = np.zeros((B, L, D_MODEL), np.float32)
    for b in range(B):
        acc = np.zeros((L, D_MODEL), np.float32)
        for hg in range(groups_per_batch):
            acc += res.results[b * groups_per_batch + hg]["partial"]
        out[b] = acc + inputs["bo"][None, :]
    return out, res


def kernel(**inputs) -> np.ndarray:
    out, _ = run(inputs)
    return out



# revision 2
# speedup vs baseline: 1.6633x; 1.6633x over previous
"""Multi-head attention (B=2, L=2048, D=1024, H=16) on 8 trn2 cores.

Sharding: core c -> (batch b = c//4) x (head-group hg = c%4, 4 heads each).
W_q/W_k/W_v are column-split, W_o row-split; the 4 partial outputs per
batch are summed on the host (plus bo).

Key compaction: the mask is key-only ([B,1,Lk]), so masked keys are
dropped on the host before upload.  K/V projections, scores, exp and
attn@V all run on the compacted LkP keys (padded to a multiple of 128;
pad slots carry a -30000 bias so exp underflows to exactly 0).

All matmul operands are bf16 (PSUM accumulation stays fp32); per-core
engine budget is balanced so ACT runs only the exp activations, DVE does
every PSUM evacuation + softmax normalization, PE does matmuls:
  K proj   KT[128,CT,LkP] = (Wk chunk).T @ xkT    (dc-inner, 2 psum bufs)
  V proj   v_all[Lk,h,65] = (xvT chunk).T @ Wv    (bias via K=1 opener mm;
                                                   col 64 = ones for denom)
  per Lq bank lb (512 cols):
    Q proj QT[:,ct,lb]    = (Wq chunk).T @ xqT
    per head: ST[128,512] = KT_c.T @ QT_b         (K = dk = 64)
              P = Exp(ST/sqrt(dk) + mask_bias)    (ACT, one op)
              OT[65,512] += v_all_c.T @ P         (row 64 = denominator)
              OT_n = OT[0:64] * bcast(1/OT[64])   (DVE recip + PE ones-mm)
    out proj out[t,half]  = sum_g OT_n[g,t].T @ Wo[g,half]  -> bf16 -> DRAM
Zero "primer" matmuls open psum accumulation groups so real matmuls never
carry more than one hardware sync-wait (walrus single-wait budget).
"""

import math
import sys

for _p in ("/opt/trn_rl_repo",):
    if _p not in sys.path:
        sys.path.insert(0, _p)

import numpy as np

import concourse.bass as bass
import concourse.mybir as mybir
import concourse.tile as tile
from concourse import bacc
from concourse.bass import ts
from concourse.bass_utils import run_bass_kernel_spmd
from concourse.tile_rust import add_dep_helper

F32 = mybir.dt.float32

D_MODEL = 1024
NUM_HEADS = 16
D_K = 64
B = 2
L = 2048
N_CORES = 8
HPC = NUM_HEADS // 4  # heads per core (4)
C = HPC * D_K         # attention columns per core (256)
CT = C // 128         # col tile groups (2)
DC = D_MODEL // 128   # d_model chunks (8)
LB = L // 512         # Lq banks (4)
SCALE = float(np.sqrt(D_K))
MASK_BIAS = -30000.0


def build_nc(LkP, L=L, D=D_MODEL, mm_dtype=mybir.dt.bfloat16):
    """Per-core Bass program (SPMD, 8 cores) for LkP compacted keys."""
    DK = D_K
    LTk = LkP // 128          # key tiles
    KB = (LkP + 511) // 512   # xk/xv 512-wide load blocks
    MDT = mm_dtype
    Ident = mybir.ActivationFunctionType.Identity

    nc = bacc.Bacc("TRN2", target_bir_lowering=False, debug=False,
                   num_devices=N_CORES)

    xqT = nc.dram_tensor("xqT", [D, L], MDT, kind="ExternalInput").ap()
    xkT = nc.dram_tensor("xkT", [D, LkP], MDT, kind="ExternalInput").ap()
    xvT = nc.dram_tensor("xvT", [D, LkP], MDT, kind="ExternalInput").ap()
    w = {n: nc.dram_tensor(f"w{n}", [D, C], MDT, kind="ExternalInput").ap()
         for n in ("q", "k", "v")}
    wo = nc.dram_tensor("wo", [C, D], MDT, kind="ExternalInput").ap()
    bias = {n: nc.dram_tensor(f"b{n}", [C], F32, kind="ExternalInput").ap()
            for n in ("q", "k")}
    bvr = nc.dram_tensor("bvr", [1, C], MDT, kind="ExternalInput").ap()
    mb = nc.dram_tensor("mb", [128, LTk], F32, kind="ExternalInput").ap()
    onesd = nc.dram_tensor("ones", [1, 128], MDT, kind="ExternalInput").ap()
    ztd = nc.dram_tensor("ztc", [128, 128], MDT, kind="ExternalInput").ap()
    vod = nc.dram_tensor("vones", [128, LTk * HPC], MDT,
                         kind="ExternalInput").ap()
    out = nc.dram_tensor("partial", [L, D], MDT, kind="ExternalOutput").ap()

    with tile.TileContext(nc) as tc:
        with (
            tc.tile_pool(name="consts", bufs=1) as consts,
            tc.tile_pool(name="persist", bufs=1) as persist,
            tc.tile_pool(name="xch", bufs=4) as xch,
            tc.tile_pool(name="work", bufs=4) as work,
            tc.tile_pool(name="norm", bufs=2) as normp,
            tc.tile_pool(name="ostg", bufs=8) as ostgp,
            tc.tile_pool(name="ps", bufs=8, space="PSUM") as psp,
        ):
            def ps_tile(name=None):
                return psp.tile([128, 512], F32, tag="ps", name=name or "pst")

            # ---- constants (SWDGE; x/out stay on the HWDGE rings) ----
            w_sb = {}
            for n in ("q", "k", "v"):
                w_sb[n] = consts.tile([128, DC, C], MDT, tag=f"w{n}",
                                      name=f"w{n}_sb")
                nc.gpsimd.dma_start(
                    out=w_sb[n], in_=w[n].rearrange("(c p) n -> p c n", p=128))
            wo_sb = consts.tile([128, CT, D], MDT, tag="wo")
            nc.gpsimd.dma_start(
                out=wo_sb, in_=wo.rearrange("(g p) n -> p g n", p=128))
            b_sb = {}
            for n in ("q", "k"):
                b_sb[n] = consts.tile([128, CT], F32, tag=f"b{n}",
                                      name=f"b{n}_sb")
                nc.gpsimd.dma_start(
                    out=b_sb[n], in_=bias[n].rearrange("(t p) -> p t", p=128))
            bvr_sb = consts.tile([1, C], MDT, tag="bvr")
            nc.gpsimd.dma_start(out=bvr_sb, in_=bvr)
            mb_sb = consts.tile([128, LTk], F32, tag="mb")
            nc.gpsimd.dma_start(out=mb_sb, in_=mb)
            ones_sb = consts.tile([1, 128], MDT, tag="ones")
            nc.gpsimd.dma_start(out=ones_sb, in_=onesd)
            zt = consts.tile([128, 128], MDT, tag="zt")
            nc.gpsimd.dma_start(out=zt, in_=ztd)
            v_all = persist.tile([128, LTk, HPC, DK + 1], MDT, tag="vall")
            nc.gpsimd.dma_start(
                out=v_all[:, :, :, DK],
                in_=vod.rearrange("p (c h) -> p c h", c=LTk))

            # const primers: make PE observe each const-DMA semaphore once
            for l_, r_ in ((w_sb["q"][:, 0, 0:1], w_sb["q"][:, 0:2, :]),
                           (w_sb["k"][:, 0, 0:1], w_sb["k"][:, 0:2, :]),
                           (w_sb["v"][:, 0, 0:1], w_sb["v"][:, 0:2, :]),
                           (wo_sb[:, 0, 0:1], wo_sb[:, 0, 0:512]),
                           (zt[:, 0:1], zt),
                           (ones_sb[:, 0:1], ones_sb),
                           (ones_sb[:, 0:1], bvr_sb),
                           (v_all[:, 0, 0, DK:DK + 1], v_all[:, :, :, DK])):
                scr = ps_tile(name="scr")
                nc.tensor.matmul(scr[0:l_.free_size(), 0:r_.free_size()],
                                 lhsT=l_, rhs=r_, start=True, stop=True,
                                 skip_group_check=True)

            # flattened const view used as the >=512-wide primer rhs
            w_rs = w_sb["q"].rearrange("p c n -> p (c n)")

            def prime(out_ps, rhs_ap, lhsT=None):
                return nc.tensor.matmul(
                    out_ps, lhsT=(lhsT if lhsT is not None else zt),
                    rhs=rhs_ap, start=True, stop=False)

            # ---- x loads (HWDGE ring, order-chained) ----
            last_xdma = None

            def xload(src, j, wdt):
                nonlocal last_xdma
                xb = xch.tile([128, DC, 512], MDT, tag="xb", name="xb")
                xd = nc.sync.dma_start(
                    out=xb[:, :, :wdt],
                    in_=src[:, ts(j, 512) if wdt == 512
                            else slice(j * 512, j * 512 + wdt)].rearrange(
                                "(c p) n -> p c n", p=128))
                if last_xdma is not None:
                    add_dep_helper(xd.ins, last_xdma.ins, sync=False,
                                   reason="xdma-order")
                last_xdma = xd
                return xb

            xkb = [xload(xkT, j, min(512, LkP - j * 512)) for j in range(KB)]
            xvb = [xload(xvT, j, min(512, LkP - j * 512)) for j in range(KB)]
            xqb = [xload(xqT, j, 512) for j in range(LB)]

            # ---- K projection: KT [128, CT, LkP] ----
            KT = persist.tile([128, CT, LkP], MDT, tag="kt")
            for j in range(KB):
                wdt = min(512, LkP - j * 512)
                for g in range(CT):
                    ps = ps_tile(name="kps")
                    prime(ps[:, :wdt], w_rs[:, :wdt])
                    for dc in range(DC):
                        nc.tensor.matmul(
                            ps[:, :wdt],
                            lhsT=w_sb["k"][:, dc, ts(g, 128)],
                            rhs=xkb[j][:, dc, :wdt],
                            start=False, stop=(dc == DC - 1))
                    nc.vector.tensor_scalar_add(
                        KT[:, g, j * 512:j * 512 + wdt], ps[:, :wdt],
                        b_sb["k"][:, g:g + 1])

            # ---- V projection: v_all [128(Lk), LTk, HPC, DK+1] ----
            for c in range(LTk):
                j, o = c // 4, (c % 4) * 128
                ps = ps_tile(name="vps")
                nc.tensor.matmul(ps[:, :C], lhsT=ones_sb, rhs=bvr_sb,
                                 start=True, stop=False)
                for dc in range(DC):
                    nc.tensor.matmul(
                        ps[:, :C],
                        lhsT=xvb[j][:, dc, o:o + 128],
                        rhs=w_sb["v"][:, dc, :],
                        start=False, stop=(dc == DC - 1))
                nc.vector.tensor_copy(
                    out=v_all[:, c, :, 0:DK],
                    in_=ps[:, :C].rearrange("p (h d) -> p h d", h=HPC))

            # ---- per-Lq-bank pipeline ----
            QT = persist.tile([128, CT, L], MDT, tag="qt")
            ot_sb = persist.tile([128, CT, L], MDT, tag="ot")
            for lb in range(LB):
                # Q projection for this bank
                for g in range(CT):
                    ps = ps_tile(name="qps")
                    prime(ps, w_rs[:, :512])
                    for dc in range(DC):
                        nc.tensor.matmul(
                            ps,
                            lhsT=w_sb["q"][:, dc, ts(g, 128)],
                            rhs=xqb[lb][:, dc, :],
                            start=False, stop=(dc == DC - 1))
                    nc.vector.tensor_scalar_add(
                        QT[:, g, ts(lb, 512)], ps,
                        b_sb["q"][:, g:g + 1])

                # attention: 4 heads interleaved, scores kept transposed
                ot_ps = [ps_tile(name=f"ot_ps_{lb}_{h}") for h in range(HPC)]
                last_prime = None
                for h in range(HPC):
                    pp = prime(ot_ps[h][:DK + 1, :], QT[:, 0, ts(lb, 512)],
                               lhsT=zt[:, :DK + 1])
                    if last_prime is not None:
                        add_dep_helper(pp.ins, last_prime.ins, sync=False,
                                       reason="prime-order")
                    last_prime = pp
                for c in range(LTk):
                    for h in range(HPC):
                        g, po = h // 2, 64 * (h % 2)
                        s_ps = ps_tile(name="s_ps")
                        smi = nc.tensor.matmul(
                            s_ps,
                            lhsT=KT[po:po + DK, g, ts(c, 128)],
                            rhs=QT[po:po + DK, g, ts(lb, 512)],
                            start=True, stop=True)
                        add_dep_helper(smi.ins, last_prime.ins, sync=False,
                                       reason="s-after-prime")
                        p_t = work.tile([128, 512], MDT, tag="p")
                        nc.scalar.activation(
                            p_t, s_ps, mybir.ActivationFunctionType.Exp,
                            bias=mb_sb[:, c:c + 1], scale=1.0 / SCALE)
                        nc.tensor.matmul(
                            ot_ps[h][:DK + 1, :],
                            lhsT=v_all[:, c, h, :],
                            rhs=p_t,
                            start=False, stop=(c == LTk - 1))
                # softmax normalization (DVE + PE ones-broadcast)
                for h in range(HPC):
                    g, po = h // 2, 64 * (h % 2)
                    rc = normp.tile([1, 512], MDT, tag="rc")
                    with nc.allow_low_precision(
                            reason="bf16 reciprocal feeds the broadcast "
                                   "matmul; |denom|>=1"):
                        nc.vector.reciprocal(rc, ot_ps[h][DK:DK + 1, :])
                    ps_rep = ps_tile(name="ps_rep")
                    nc.tensor.matmul(ps_rep[:DK, :], lhsT=ones_sb[:, :DK],
                                     rhs=rc, start=True, stop=True)
                    rep = normp.tile([64, 512], F32, tag="rep")
                    nc.vector.tensor_copy(out=rep, in_=ps_rep[:DK, :])
                    nc.vector.tensor_mul(
                        ot_sb[po:po + DK, g, ts(lb, 512)],
                        ot_ps[h][0:DK, :], rep)

                # output projection for this bank's 4 Lq tiles
                for tt in range(4):
                    t = lb * 4 + tt
                    for half in range(2):
                        wps = ps_tile(name="wps")
                        for g in range(CT):
                            nc.tensor.matmul(
                                wps,
                                lhsT=ot_sb[:, g, ts(t, 128)],
                                rhs=wo_sb[:, g, ts(half, 512)],
                                start=(g == 0), stop=(g == CT - 1))
                        og = ostgp.tile([128, 512], MDT, tag="os", name="ostg")
                        nc.vector.tensor_copy(out=og, in_=wps)
                        od = nc.sync.dma_start(
                            out=out[ts(t, 128), ts(half, 512)], in_=og)
                        add_dep_helper(od.ins, last_xdma.ins, sync=False,
                                       reason="odma-order")
                        last_xdma = od

    nc.compile()
    _strip_implied_dma_ring_waits(nc)
    return nc


def _strip_implied_dma_ring_waits(nc):
    """Drop DMA ring-semaphore waits implied by a compute-engine wait on the
    same descriptor (DMA descriptors carry a single hardware sync-wait).
    Applied to the x-block loads and output-staging stores, whose only
    DMA-semaphore deps are WAW/WAR-release edges already covered by the
    readers' engine semaphore."""
    import concourse.mybir as _mb
    for ins in nc.inst_map.values():
        if type(ins).__name__ != "InstDMACopy":
            continue
        if not ins.outs:
            continue
        memref = getattr(ins.outs[0], "memref", "") or ""
        src_ref = getattr(ins.ins[0], "memref", "") if ins.ins else ""
        if not (memref.startswith("xb") or (src_ref or "").startswith("ostg")):
            continue
        si = ins.sync_info
        if not si or not si.on_wait or len(si.on_wait) < 2:
            continue
        eng = [w_ for w_ in si.on_wait
               if not (w_.ant_name or "").startswith(("DMAHW", "DMASW"))]
        if not eng:
            continue
        ins.sync_info = _mb.SyncInfo(on_wait=eng, on_update=list(si.on_update))


def make_in_maps(query, key, value, mask, Wq, bq, Wk, bk, Wv, bv, Wo, bo,
                 LkP, mm_dtype=mybir.dt.bfloat16):
    """Host-side sharding + key compaction: per-core input dicts."""
    DK = D_K
    LTk = LkP // 128
    mdt = mybir.dt.np(mm_dtype)
    ones = np.ones((1, 128), np.float32).astype(mdt)
    ztc = np.zeros((128, 128), np.float32).astype(mdt)
    vones = np.ones((128, LTk * HPC), np.float32).astype(mdt)
    in_maps = []
    xTs, mbs = {}, {}
    for b in range(B):
        keep = np.flatnonzero(~mask[b, 0])
        n = len(keep)
        xkc = np.zeros((D_MODEL, LkP), mdt)
        xvc = np.zeros((D_MODEL, LkP), mdt)
        xkc[:, :n] = key[b].T[:, keep].astype(mdt)
        xvc[:, :n] = value[b].T[:, keep].astype(mdt)
        xTs[b] = {
            "q": np.ascontiguousarray(query[b].T.astype(mdt)),
            "k": xkc,
            "v": xvc,
        }
        mbf = np.full(LkP, np.float32(MASK_BIAS))
        mbf[:n] = 0.0
        mbs[b] = np.ascontiguousarray(mbf.reshape(LTk, 128).T)
    for c in range(N_CORES):
        b, hg = divmod(c, N_CORES // B)
        sl = slice(hg * C, (hg + 1) * C)
        in_maps.append({
            "xqT": xTs[b]["q"], "xkT": xTs[b]["k"], "xvT": xTs[b]["v"],
            "wq": np.ascontiguousarray(Wq[:, sl].astype(mdt)),
            "wk": np.ascontiguousarray(Wk[:, sl].astype(mdt)),
            "wv": np.ascontiguousarray(Wv[:, sl].astype(mdt)),
            "wo": np.ascontiguousarray(Wo[sl, :].astype(mdt)),
            "bq": np.ascontiguousarray(bq[sl].astype(np.float32)),
            "bk": np.ascontiguousarray(bk[sl].astype(np.float32)),
            "bvr": np.ascontiguousarray(bv[sl].astype(mdt)[None, :]),
            "mb": mbs[b],
            "ones": ones, "ztc": ztc, "vones": vones,
        })
    return in_maps


_NC_CACHE = {}


def _get_nc(LkP, mm_dtype=mybir.dt.bfloat16):
    key = (str(mm_dtype), LkP)
    if key not in _NC_CACHE:
        _NC_CACHE[key] = build_nc(LkP, mm_dtype=mm_dtype)
    return _NC_CACHE[key]


def run(inputs, mm_dtype=mybir.dt.bfloat16, trace=False):
    """Run on 8 cores; returns (full_output, BassKernelResults)."""
    inputs = {k: np.asarray(v) for k, v in inputs.items()}
    mask = inputs["mask"]
    counts = [int((~mask[b, 0]).sum()) for b in range(B)]
    LkP = max(128, 128 * int(math.ceil(max(counts) / 128.0)))
    nc = _get_nc(LkP, mm_dtype)
    in_maps = make_in_maps(**inputs, LkP=LkP, mm_dtype=mm_dtype)
    res = run_bass_kernel_spmd(nc, in_maps, list(range(N_CORES)), trace=trace)
    groups_per_batch = N_CORES // B
    out = np.zeros((B, L, D_MODEL), np.float32)
    for b in range(B):
        acc = np.zeros((L, D_MODEL), np.float32)
        if counts[b] > 0:
            for hg in range(groups_per_batch):
                acc += np.asarray(
                    res.results[b * groups_per_batch + hg]["partial"]
                ).astype(np.float32)
        out[b] = acc + inputs["bo"][None, :]
    return out, res


def kernel(**inputs) -> np.ndarray:
    out, _ = run(inputs)
    return out


# revision 3
# speedup vs baseline: 1.8400x; 1.1063x over previous
"""Multi-head attention (B=2, L=2048, D=1024, H=16) on 8 trn2 cores.

Sharding: core c -> (batch b = c//4) x (head-group hg = c%4, 4 heads each).
W_q/W_k/W_v are column-split, W_o row-split; the 4 partial outputs per
batch are summed on the host (plus bo).

Key compaction: the mask is key-only ([B,1,Lk]), so masked keys are
dropped on the host before upload.  K/V projections, scores, exp and
attn@V all run on the compacted LkP keys (padded to a multiple of 128;
pad slots carry a -30000 bias so exp underflows to exactly 0).

All matmul operands are bf16 (PSUM accumulation stays fp32); per-core
engine budget is balanced so ACT runs only the exp activations, DVE does
every PSUM evacuation + softmax normalization, PE does matmuls:
  K proj   KT[128,CT,LkP] = (Wk chunk).T @ xkT    (dc-inner, 2 psum bufs)
  V proj   v_all[Lk,h,65] = (xvT chunk).T @ Wv    (bias via K=1 opener mm;
                                                   col 64 = ones for denom)
  per Lq bank lb (512 cols):
    Q proj QT[:,ct,lb]    = (Wq chunk).T @ xqT
    per head: ST[128,512] = KT_c.T @ QT_b         (K = dk = 64)
              P = Exp(ST/sqrt(dk) + mask_bias)    (ACT, one op)
              OT[65,512] += v_all_c.T @ P         (row 64 = denominator)
              OT_n = OT[0:64] * bcast(1/OT[64])   (DVE recip + PE ones-mm)
    out proj out[t,half]  = sum_g OT_n[g,t].T @ Wo[g,half]  -> bf16 -> DRAM
Zero "primer" matmuls open psum accumulation groups so real matmuls never
carry more than one hardware sync-wait (walrus single-wait budget).
"""

import math
import sys

for _p in ("/opt/trn_rl_repo",):
    if _p not in sys.path:
        sys.path.insert(0, _p)

import numpy as np

import concourse.bass as bass
import concourse.mybir as mybir
import concourse.tile as tile
from concourse import bacc
from concourse.bass import ts
from concourse.bass_utils import run_bass_kernel_spmd
from concourse.tile_rust import add_dep_helper

F32 = mybir.dt.float32

D_MODEL = 1024
NUM_HEADS = 16
D_K = 64
B = 2
L = 2048
N_CORES = 8
HPC = NUM_HEADS // 4  # heads per core (4)
C = HPC * D_K         # attention columns per core (256)
CT = C // 128         # col tile groups (2)
DC = D_MODEL // 128   # d_model chunks (8)
LB = L // 512         # Lq banks (4)
SCALE = float(np.sqrt(D_K))
MASK_BIAS = -30000.0


def build_nc(LkP, L=L, D=D_MODEL, mm_dtype=mybir.dt.bfloat16):
    """Per-core Bass program (SPMD, 8 cores) for LkP compacted keys."""
    DK = D_K
    LTk = LkP // 128          # key tiles
    KB = (LkP + 511) // 512   # xk/xv 512-wide load blocks
    MDT = mm_dtype
    Ident = mybir.ActivationFunctionType.Identity

    nc = bacc.Bacc("TRN2", target_bir_lowering=False, debug=False,
                   num_devices=N_CORES)

    xqT = nc.dram_tensor("xqT", [D, L], MDT, kind="ExternalInput").ap()
    xkT = nc.dram_tensor("xkT", [D, LkP], MDT, kind="ExternalInput").ap()
    xvT = nc.dram_tensor("xvT", [D, LkP], MDT, kind="ExternalInput").ap()
    w = {n: nc.dram_tensor(f"w{n}", [D, C], MDT, kind="ExternalInput").ap()
         for n in ("q", "k", "v")}
    wo = nc.dram_tensor("wo", [C, D], MDT, kind="ExternalInput").ap()
    bias = {n: nc.dram_tensor(f"b{n}", [C], F32, kind="ExternalInput").ap()
            for n in ("q", "k")}
    bvr = nc.dram_tensor("bvr", [1, C], MDT, kind="ExternalInput").ap()
    mb = nc.dram_tensor("mb", [128, LTk], F32, kind="ExternalInput").ap()
    onesd = nc.dram_tensor("ones", [1, 128], MDT, kind="ExternalInput").ap()
    ztd = nc.dram_tensor("ztc", [128, 128], MDT, kind="ExternalInput").ap()
    vod = nc.dram_tensor("vones", [128, LTk * HPC], MDT,
                         kind="ExternalInput").ap()
    out = nc.dram_tensor("partial", [L, D], MDT, kind="ExternalOutput").ap()

    with tile.TileContext(nc) as tc:
        with (
            tc.tile_pool(name="consts", bufs=1) as consts,
            tc.tile_pool(name="persist", bufs=1) as persist,
            tc.tile_pool(name="xch", bufs=4) as xch,
            tc.tile_pool(name="work", bufs=4) as work,
            tc.tile_pool(name="norm", bufs=2) as normp,
            tc.tile_pool(name="ostg", bufs=8) as ostgp,
            tc.tile_pool(name="ps", bufs=8, space="PSUM") as psp,
        ):
            def ps_tile(name=None):
                return psp.tile([128, 512], F32, tag="ps", name=name or "pst")

            # ---- constants (SWDGE; x/out stay on the HWDGE rings) ----
            w_sb = {}
            for n in ("q", "k", "v"):
                w_sb[n] = consts.tile([128, DC, C], MDT, tag=f"w{n}",
                                      name=f"w{n}_sb")
                nc.gpsimd.dma_start(
                    out=w_sb[n], in_=w[n].rearrange("(c p) n -> p c n", p=128))
            wo_sb = consts.tile([128, CT, D], MDT, tag="wo")
            nc.gpsimd.dma_start(
                out=wo_sb, in_=wo.rearrange("(g p) n -> p g n", p=128))
            b_sb = {}
            for n in ("q", "k"):
                b_sb[n] = consts.tile([128, CT], F32, tag=f"b{n}",
                                      name=f"b{n}_sb")
                nc.gpsimd.dma_start(
                    out=b_sb[n], in_=bias[n].rearrange("(t p) -> p t", p=128))
            bvr_sb = consts.tile([1, C], MDT, tag="bvr")
            nc.gpsimd.dma_start(out=bvr_sb, in_=bvr)
            mb_sb = consts.tile([128, LTk], F32, tag="mb")
            nc.gpsimd.dma_start(out=mb_sb, in_=mb)
            ones_sb = consts.tile([1, 128], MDT, tag="ones")
            nc.gpsimd.dma_start(out=ones_sb, in_=onesd)
            zt = consts.tile([128, 128], MDT, tag="zt")
            nc.gpsimd.dma_start(out=zt, in_=ztd)
            v_all = persist.tile([128, LTk, HPC, DK + 1], MDT, tag="vall")
            nc.gpsimd.dma_start(
                out=v_all[:, :, :, DK],
                in_=vod.rearrange("p (c h) -> p c h", c=LTk))

            # const primers: make PE observe each const-DMA semaphore once
            for l_, r_ in ((w_sb["q"][:, 0, 0:1], w_sb["q"][:, 0:2, :]),
                           (w_sb["k"][:, 0, 0:1], w_sb["k"][:, 0:2, :]),
                           (w_sb["v"][:, 0, 0:1], w_sb["v"][:, 0:2, :]),
                           (wo_sb[:, 0, 0:1], wo_sb[:, 0, 0:512]),
                           (zt[:, 0:1], zt),
                           (ones_sb[:, 0:1], ones_sb),
                           (ones_sb[:, 0:1], bvr_sb),
                           (v_all[:, 0, 0, DK:DK + 1], v_all[:, :, :, DK])):
                scr = ps_tile(name="scr")
                nc.tensor.matmul(scr[0:l_.free_size(), 0:r_.free_size()],
                                 lhsT=l_, rhs=r_, start=True, stop=True,
                                 skip_group_check=True)

            # flattened const view used as the >=512-wide primer rhs
            w_rs = w_sb["q"].rearrange("p c n -> p (c n)")

            def prime(out_ps, rhs_ap, lhsT=None):
                return nc.tensor.matmul(
                    out_ps, lhsT=(lhsT if lhsT is not None else zt),
                    rhs=rhs_ap, start=True, stop=False)

            # ---- x loads (HWDGE ring, order-chained) ----
            last_xdma = None

            def xload(src, j, wdt):
                nonlocal last_xdma
                xb = xch.tile([128, DC, 512], MDT, tag="xb", name="xb")
                xd = nc.sync.dma_start(
                    out=xb[:, :, :wdt],
                    in_=src[:, ts(j, 512) if wdt == 512
                            else slice(j * 512, j * 512 + wdt)].rearrange(
                                "(c p) n -> p c n", p=128))
                if last_xdma is not None:
                    add_dep_helper(xd.ins, last_xdma.ins, sync=False,
                                   reason="xdma-order")
                last_xdma = xd
                return xb

            xkb = [xload(xkT, j, min(512, LkP - j * 512)) for j in range(KB)]
            xvb = [xload(xvT, j, min(512, LkP - j * 512)) for j in range(KB)]
            xqb = [xload(xqT, j, 512) for j in range(LB)]

            # ---- K projection: KT [128, CT, LkP] ----
            KT = persist.tile([128, CT, LkP], MDT, tag="kt")
            for j in range(KB):
                wdt = min(512, LkP - j * 512)
                for g in range(CT):
                    ps = ps_tile(name="kps")
                    prime(ps[:, :wdt], w_rs[:, :wdt])
                    for dc in range(DC):
                        nc.tensor.matmul(
                            ps[:, :wdt],
                            lhsT=w_sb["k"][:, dc, ts(g, 128)],
                            rhs=xkb[j][:, dc, :wdt],
                            start=False, stop=(dc == DC - 1))
                    nc.vector.tensor_scalar_add(
                        KT[:, g, j * 512:j * 512 + wdt], ps[:, :wdt],
                        b_sb["k"][:, g:g + 1])

            # ---- V projection: v_all [128(Lk), LTk, HPC, DK+1] ----
            for c in range(LTk):
                j, o = c // 4, (c % 4) * 128
                ps = ps_tile(name="vps")
                nc.tensor.matmul(ps[:, :C], lhsT=ones_sb, rhs=bvr_sb,
                                 start=True, stop=False)
                for dc in range(DC):
                    nc.tensor.matmul(
                        ps[:, :C],
                        lhsT=xvb[j][:, dc, o:o + 128],
                        rhs=w_sb["v"][:, dc, :],
                        start=False, stop=(dc == DC - 1))
                nc.vector.tensor_copy(
                    out=v_all[:, c, :, 0:DK],
                    in_=ps[:, :C].rearrange("p (h d) -> p h d", h=HPC))

            # ---- per-Lq-bank software pipeline ----
            # bank lb's exp-gated c-loop absorbs, as background PE work,
            # the Q projection of bank lb+1 and the output projection of
            # bank lb-1, so PE never stalls on the in-order DVE stream.
            QT = persist.tile([128, CT, L], MDT, tag="qt")
            ot_sb = persist.tile([128, CT, L], MDT, tag="ot")

            def qproj_ops(lb):
                """Closure list: project QT[:, :, lb*512 +: 512]."""
                ops = []
                for g in range(CT):
                    ps = ps_tile(name="qps")

                    def op_prime(ps=ps):
                        prime(ps, w_rs[:, :512])
                    ops.append(op_prime)
                    for dc in range(DC):
                        def op_mm(ps=ps, g=g, dc=dc):
                            nc.tensor.matmul(
                                ps,
                                lhsT=w_sb["q"][:, dc, ts(g, 128)],
                                rhs=xqb[lb][:, dc, :],
                                start=False, stop=(dc == DC - 1))
                        ops.append(op_mm)

                    def op_evac(ps=ps, g=g):
                        nc.vector.tensor_scalar_add(
                            QT[:, g, ts(lb, 512)], ps, b_sb["q"][:, g:g + 1])
                    ops.append(op_evac)
                return ops

            def oproj_ops(lb):
                """Closure list: project out rows lb*512 +: 512."""
                ops = []
                for tt in range(4):
                    t = lb * 4 + tt
                    for half in range(2):
                        wps = ps_tile(name="wps")
                        for g in range(CT):
                            def op_mm(wps=wps, t=t, half=half, g=g):
                                nc.tensor.matmul(
                                    wps,
                                    lhsT=ot_sb[:, g, ts(t, 128)],
                                    rhs=wo_sb[:, g, ts(half, 512)],
                                    start=(g == 0), stop=(g == CT - 1))
                            ops.append(op_mm)

                        def op_stage(wps=wps, t=t, half=half):
                            og = ostgp.tile([128, 512], MDT, tag="os",
                                            name="ostg")
                            nc.vector.tensor_copy(out=og, in_=wps)
                            od = nc.sync.dma_start(
                                out=out[ts(t, 128), ts(half, 512)], in_=og)
                            add_dep_helper(od.ins, odma[0].ins, sync=False,
                                           reason="odma-order")
                            odma[0] = od
                        ops.append(op_stage)
                return ops

            odma = [last_xdma]
            for op in qproj_ops(0):
                op()
            for lb in range(LB):
                bg = []
                if lb + 1 < LB:
                    bg += qproj_ops(lb + 1)
                if lb > 0:
                    bg += oproj_ops(lb - 1)

                # attention: 4 heads interleaved, scores kept transposed
                ot_ps = [ps_tile(name=f"ot_ps_{lb}_{h}") for h in range(HPC)]
                last_prime = None
                for h in range(HPC):
                    pp = prime(ot_ps[h][:DK + 1, :], QT[:, 0, ts(lb, 512)],
                               lhsT=zt[:, :DK + 1])
                    if last_prime is not None:
                        add_dep_helper(pp.ins, last_prime.ins, sync=False,
                                       reason="prime-order")
                    last_prime = pp
                for c in range(LTk):
                    for h in range(HPC):
                        g, po = h // 2, 64 * (h % 2)
                        s_ps = ps_tile(name="s_ps")
                        smi = nc.tensor.matmul(
                            s_ps,
                            lhsT=KT[po:po + DK, g, ts(c, 128)],
                            rhs=QT[po:po + DK, g, ts(lb, 512)],
                            start=True, stop=True)
                        add_dep_helper(smi.ins, last_prime.ins, sync=False,
                                       reason="s-after-prime")
                        p_t = work.tile([128, 512], MDT, tag="p")
                        nc.scalar.activation(
                            p_t, s_ps, mybir.ActivationFunctionType.Exp,
                            bias=mb_sb[:, c:c + 1], scale=1.0 / SCALE)
                        nc.tensor.matmul(
                            ot_ps[h][:DK + 1, :],
                            lhsT=v_all[:, c, h, :],
                            rhs=p_t,
                            start=False, stop=(c == LTk - 1))
                    take = -(-len(bg) // (LTk - c))  # spread over c-iters
                    for op in bg[:take]:
                        op()
                    bg = bg[take:]

                # softmax normalization, engine-phased so the per-head
                # recip -> bcast -> rep -> mult chains pipeline across
                # DVE / PE instead of serializing head by head
                rcs, reps = [], []
                for h in range(HPC):
                    rc = normp.tile([1, 512], MDT, tag="rc", bufs=4)
                    with nc.allow_low_precision(
                            reason="bf16 reciprocal feeds the broadcast "
                                   "matmul; |denom|>=1"):
                        nc.vector.reciprocal(rc, ot_ps[h][DK:DK + 1, :])
                    rcs.append(rc)
                for h in range(HPC):
                    ps_rep = ps_tile(name="ps_rep")
                    nc.tensor.matmul(ps_rep[:DK, :], lhsT=ones_sb[:, :DK],
                                     rhs=rcs[h], start=True, stop=True)
                    reps.append(ps_rep)
                rsb = []
                for h in range(HPC):
                    rep = normp.tile([64, 512], F32, tag="rep", bufs=4)
                    nc.vector.tensor_copy(out=rep, in_=reps[h][:DK, :])
                    rsb.append(rep)
                for h in range(HPC):
                    g, po = h // 2, 64 * (h % 2)
                    nc.vector.tensor_mul(
                        ot_sb[po:po + DK, g, ts(lb, 512)],
                        ot_ps[h][0:DK, :], rsb[h])

            # last bank's output projection
            for op in oproj_ops(LB - 1):
                op()

    nc.compile()
    _strip_implied_dma_ring_waits(nc)
    return nc


def _strip_implied_dma_ring_waits(nc):
    """Drop DMA ring-semaphore waits implied by a compute-engine wait on the
    same descriptor (DMA descriptors carry a single hardware sync-wait).
    Applied to the x-block loads and output-staging stores, whose only
    DMA-semaphore deps are WAW/WAR-release edges already covered by the
    readers' engine semaphore."""
    import concourse.mybir as _mb
    for ins in nc.inst_map.values():
        if type(ins).__name__ != "InstDMACopy":
            continue
        if not ins.outs:
            continue
        memref = getattr(ins.outs[0], "memref", "") or ""
        src_ref = getattr(ins.ins[0], "memref", "") if ins.ins else ""
        if not (memref.startswith("xb") or (src_ref or "").startswith("ostg")):
            continue
        si = ins.sync_info
        if not si or not si.on_wait or len(si.on_wait) < 2:
            continue
        eng = [w_ for w_ in si.on_wait
               if not (w_.ant_name or "").startswith(("DMAHW", "DMASW"))]
        if not eng:
            continue
        ins.sync_info = _mb.SyncInfo(on_wait=eng, on_update=list(si.on_update))


def make_in_maps(query, key, value, mask, Wq, bq, Wk, bk, Wv, bv, Wo, bo,
                 LkP, mm_dtype=mybir.dt.bfloat16):
    """Host-side sharding + key compaction: per-core input dicts."""
    DK = D_K
    LTk = LkP // 128
    mdt = mybir.dt.np(mm_dtype)
    ones = np.ones((1, 128), np.float32).astype(mdt)
    ztc = np.zeros((128, 128), np.float32).astype(mdt)
    vones = np.ones((128, LTk * HPC), np.float32).astype(mdt)
    in_maps = []
    xTs, mbs = {}, {}
    for b in range(B):
        keep = np.flatnonzero(~mask[b, 0])
        n = len(keep)
        xkc = np.zeros((D_MODEL, LkP), mdt)
        xvc = np.zeros((D_MODEL, LkP), mdt)
        xkc[:, :n] = key[b].T[:, keep].astype(mdt)
        xvc[:, :n] = value[b].T[:, keep].astype(mdt)
        xTs[b] = {
            "q": np.ascontiguousarray(query[b].T.astype(mdt)),
            "k": xkc,
            "v": xvc,
        }
        mbf = np.full(LkP, np.float32(MASK_BIAS))
        mbf[:n] = 0.0
        mbs[b] = np.ascontiguousarray(mbf.reshape(LTk, 128).T)
    for c in range(N_CORES):
        b, hg = divmod(c, N_CORES // B)
        sl = slice(hg * C, (hg + 1) * C)
        in_maps.append({
            "xqT": xTs[b]["q"], "xkT": xTs[b]["k"], "xvT": xTs[b]["v"],
            "wq": np.ascontiguousarray(Wq[:, sl].astype(mdt)),
            "wk": np.ascontiguousarray(Wk[:, sl].astype(mdt)),
            "wv": np.ascontiguousarray(Wv[:, sl].astype(mdt)),
            "wo": np.ascontiguousarray(Wo[sl, :].astype(mdt)),
            "bq": np.ascontiguousarray(bq[sl].astype(np.float32)),
            "bk": np.ascontiguousarray(bk[sl].astype(np.float32)),
            "bvr": np.ascontiguousarray(bv[sl].astype(mdt)[None, :]),
            "mb": mbs[b],
            "ones": ones, "ztc": ztc, "vones": vones,
        })
    return in_maps


_NC_CACHE = {}


def _get_nc(LkP, mm_dtype=mybir.dt.bfloat16):
    key = (str(mm_dtype), LkP)
    if key not in _NC_CACHE:
        _NC_CACHE[key] = build_nc(LkP, mm_dtype=mm_dtype)
    return _NC_CACHE[key]


def run(inputs, mm_dtype=mybir.dt.bfloat16, trace=False):
    """Run on 8 cores; returns (full_output, BassKernelResults)."""
    inputs = {k: np.asarray(v) for k, v in inputs.items()}
    mask = inputs["mask"]
    counts = [int((~mask[b, 0]).sum()) for b in range(B)]
    LkP = max(128, 128 * int(math.ceil(max(counts) / 128.0)))
    nc = _get_nc(LkP, mm_dtype)
    in_maps = make_in_maps(**inputs, LkP=LkP, mm_dtype=mm_dtype)
    res = run_bass_kernel_spmd(nc, in_maps, list(range(N_CORES)), trace=trace)
    groups_per_batch = N_CORES // B
    out = np.zeros((B, L, D_MODEL), np.float32)
    for b in range(B):
        acc = np.zeros((L, D_MODEL), np.float32)
        if counts[b] > 0:
            for hg in range(groups_per_batch):
                acc += np.asarray(
                    res.results[b * groups_per_batch + hg]["partial"]
                ).astype(np.float32)
        out[b] = acc + inputs["bo"][None, :]
    return out, res


def kernel(**inputs) -> np.ndarray:
    out, _ = run(inputs)
    return out


# revision 6
# speedup vs baseline: 2.1520x; 1.1695x over previous
"""Multi-head attention (B=2, L=2048, D=1024, H=16) on 8 trn2 cores.

Sharding: core c -> (batch b = c//4) x (head-group hg = c%4, 4 heads each).
W_q/W_k/W_v are column-split, W_o row-split; the 4 partial outputs per
batch are summed on the host (plus bo).

Key compaction: the mask is key-only ([B,1,Lk]), so masked keys are
dropped on the host before upload.  K/V projections, scores, exp and
attn@V all run on the compacted LkP keys (padded to a multiple of 128;
pad slots carry a -30000 bias so exp underflows to exactly 0).

All matmul operands are bf16 (PSUM accumulation stays fp32).  Engine
budget: ACT runs only the exp activations, DVE does every PSUM
evacuation + softmax normalization, PE does matmuls.  The kernel is a
software pipeline over the four 512-wide Lq banks: bank lb's exp-gated
score/attnV loop absorbs, as background PE/DVE ops, the straggler K/V
projection tiles, the Q projection of bank lb+1, the softmax
normalization of bank lb-1 and the output projection of bank lb-2, so
no engine ever sits behind the in-order DVE queue.

  K proj   KT[128,CT,LkP] = (Wk chunk).T @ xkT    (dc-inner, psum rotate)
  V proj   v_all[Lk,h,65] = (xvT chunk).T @ Wv    (bias via K=1 opener mm;
                                                   col 64 = ones for denom)
  scores   ST[128,512]    = KT_c.T @ QT_b         (K = dk = 64)
  exp      P = Exp(ST/sqrt(dk) + mask_bias)       (ACT, one op per tile)
  attn@V   OT[65,512]    += v_all_c.T @ P         (row 64 = denominator)
  evac     otc = OT (bf16)                        (frees the psum bank)
  norm     ot_n = otc[0:64] * bcast(1/otc[64])    (DVE recip + PE ones-mm)
  out proj out[t,half]    = sum_g ot_n[g,t].T @ Wo[g,half] -> bf16 -> DRAM

Zero "primer" matmuls open psum accumulation groups so real matmuls never
carry more than one hardware sync-wait (walrus single-wait budget).
"""

import math
import sys

for _p in ("/opt/trn_rl_repo",):
    if _p not in sys.path:
        sys.path.insert(0, _p)

import numpy as np

import concourse.bass as bass
import concourse.mybir as mybir
import concourse.tile as tile
from concourse import bacc
from concourse.bass import ts
from concourse.bass_utils import run_bass_kernel_spmd
from concourse.tile_rust import add_dep_helper

F32 = mybir.dt.float32

D_MODEL = 1024
NUM_HEADS = 16
D_K = 64
B = 2
L = 2048
N_CORES = 8
HPC = NUM_HEADS // 4  # heads per core (4)
C = HPC * D_K         # attention columns per core (256)
CT = C // 128         # col tile groups (2)
DC = D_MODEL // 128   # d_model chunks (8)
LB = L // 512         # Lq banks (4)
SCALE = float(np.sqrt(D_K))
MASK_BIAS = -30000.0


def build_nc(LkP, L=L, D=D_MODEL, mm_dtype=mybir.dt.bfloat16):
    """Per-core Bass program (SPMD, 8 cores) for LkP compacted keys."""
    DK = D_K
    LTk = LkP // 128          # key tiles
    KB = (LkP + 511) // 512   # xk/xv 512-wide load blocks
    VF = min(4, LTk)          # V tiles projected in the foreground
    MDT = mm_dtype

    nc = bacc.Bacc("TRN2", target_bir_lowering=False, debug=False,
                   num_devices=N_CORES)

    xqT = nc.dram_tensor("xqT", [D, L], MDT, kind="ExternalInput").ap()
    xkT = nc.dram_tensor("xkT", [D, LkP], MDT, kind="ExternalInput").ap()
    xvT = nc.dram_tensor("xvT", [D, LkP], MDT, kind="ExternalInput").ap()
    w = {n: nc.dram_tensor(f"w{n}", [D, C], MDT, kind="ExternalInput").ap()
         for n in ("q", "k", "v")}
    wo = nc.dram_tensor("wo", [C, D], MDT, kind="ExternalInput").ap()
    bias = {n: nc.dram_tensor(f"b{n}", [C], F32, kind="ExternalInput").ap()
            for n in ("q", "k")}
    bvr = nc.dram_tensor("bvr", [1, C], MDT, kind="ExternalInput").ap()
    mb = nc.dram_tensor("mb", [128, LTk], F32, kind="ExternalInput").ap()
    onesd = nc.dram_tensor("ones", [1, 128], MDT, kind="ExternalInput").ap()
    ztd = nc.dram_tensor("ztc", [128, 128], MDT, kind="ExternalInput").ap()
    vod = nc.dram_tensor("vones", [128, LTk * HPC], MDT,
                         kind="ExternalInput").ap()
    out = nc.dram_tensor("partial", [L, D], MDT, kind="ExternalOutput").ap()

    with tile.TileContext(nc) as tc:
        with (
            tc.tile_pool(name="consts", bufs=1) as consts,
            tc.tile_pool(name="persist", bufs=1) as persist,
            tc.tile_pool(name="xch", bufs=4) as xch,
            tc.tile_pool(name="work", bufs=4) as work,
            tc.tile_pool(name="norm", bufs=4) as normp,
            tc.tile_pool(name="ostg", bufs=8) as ostgp,
            tc.tile_pool(name="ps", bufs=8, space="PSUM") as psp,
        ):
            def ps_tile(name=None):
                return psp.tile([128, 512], F32, tag="ps", name=name or "pst")

            # ---- constants (SWDGE; x/out stay on the HWDGE rings) ----
            # Emitted in deadline order: wk feeds the very first matmuls.
            w_sb, b_sb = {}, {}

            def load_w(n):
                w_sb[n] = consts.tile([128, DC, C], MDT, tag=f"w{n}",
                                      name=f"w{n}_sb")
                nc.gpsimd.dma_start(
                    out=w_sb[n], in_=w[n].rearrange("(c p) n -> p c n", p=128))

            def load_b(n):
                b_sb[n] = consts.tile([128, CT], F32, tag=f"b{n}",
                                      name=f"b{n}_sb")
                nc.gpsimd.dma_start(
                    out=b_sb[n], in_=bias[n].rearrange("(t p) -> p t", p=128))

            load_w("k")
            zt = consts.tile([128, 128], MDT, tag="zt")
            nc.gpsimd.dma_start(out=zt, in_=ztd)
            ones_sb = consts.tile([1, 128], MDT, tag="ones")
            nc.gpsimd.dma_start(out=ones_sb, in_=onesd)
            bvr_sb = consts.tile([1, C], MDT, tag="bvr")
            nc.gpsimd.dma_start(out=bvr_sb, in_=bvr)
            load_b("k")
            load_w("v")
            load_w("q")
            mb_sb = consts.tile([128, LTk], F32, tag="mb")
            nc.gpsimd.dma_start(out=mb_sb, in_=mb)
            load_b("q")
            wo_sb = consts.tile([128, CT, D], MDT, tag="wo")
            nc.gpsimd.dma_start(
                out=wo_sb, in_=wo.rearrange("(g p) n -> p g n", p=128))
            v_all = persist.tile([128, LTk, HPC, DK + 1], MDT, tag="vall")
            nc.gpsimd.dma_start(
                out=v_all[:, :, :, DK],
                in_=vod.rearrange("p (c h) -> p c h", c=LTk))

            def const_prime(l_, r_):
                scr = ps_tile(name="scr")
                nc.tensor.matmul(scr[0:l_.free_size(), 0:r_.free_size()],
                                 lhsT=l_, rhs=r_, start=True, stop=True,
                                 skip_group_check=True)

            w_rs = {n: w_sb[n].rearrange("p c n -> p (c n)")
                    for n in ("q", "k", "v")}

            def prime(out_ps, rhs_ap, lhsT=None):
                return nc.tensor.matmul(
                    out_ps, lhsT=(lhsT if lhsT is not None else zt),
                    rhs=rhs_ap, start=True, stop=False)

            # ---- x loads (HWDGE ring, order-chained, deadline order) ----
            last_dma = [None]

            def xload(src, j, wdt):
                xb = xch.tile([128, DC, 512], MDT, tag="xb", name="xb")
                xd = nc.sync.dma_start(
                    out=xb[:, :, :wdt],
                    in_=src[:, j * 512:j * 512 + wdt].rearrange(
                        "(c p) n -> p c n", p=128))
                if last_dma[0] is not None:
                    add_dep_helper(xd.ins, last_dma[0].ins, sync=False,
                                   reason="dma-order")
                last_dma[0] = xd
                return xb

            kw = [min(512, LkP - j * 512) for j in range(KB)]
            xkb, xvb, xqb = [None] * KB, [None] * KB, [None] * LB
            xkb[0] = xload(xkT, 0, kw[0])
            xqb[0] = xload(xqT, 0, 512)
            xvb[0] = xload(xvT, 0, kw[0])
            for j in range(1, KB):
                xkb[j] = xload(xkT, j, kw[j])
                xvb[j] = xload(xvT, j, kw[j])
            for j in range(1, LB):
                xqb[j] = xload(xqT, j, 512)

            # early const primers (wk / zt / ones / bvr only)
            const_prime(w_sb["k"][:, 0, 0:1], w_sb["k"][:, 0:2, :])
            const_prime(zt[:, 0:1], zt)
            const_prime(ones_sb[:, 0:1], ones_sb)
            const_prime(ones_sb[:, 0:1], bvr_sb)

            KT = persist.tile([128, CT, LkP], MDT, tag="kt")
            QT = persist.tile([128, CT, L], MDT, tag="qt")
            ot_sb = persist.tile([128, CT, L], MDT, tag="ot")

            def kproj_ops(j):
                ops = []
                for g in range(CT):
                    ps = ps_tile(name="kps")
                    ops.append(lambda ps=ps, j=j: prime(
                        ps[:, :kw[j]], w_rs["k"][:, :kw[j]]))
                    for dc in range(DC):
                        ops.append(lambda ps=ps, j=j, g=g, dc=dc:
                                   nc.tensor.matmul(
                                       ps[:, :kw[j]],
                                       lhsT=w_sb["k"][:, dc, ts(g, 128)],
                                       rhs=xkb[j][:, dc, :kw[j]],
                                       start=False, stop=(dc == DC - 1)))
                    ops.append(lambda ps=ps, j=j, g=g:
                               nc.vector.tensor_scalar_add(
                                   KT[:, g, j * 512:j * 512 + kw[j]],
                                   ps[:, :kw[j]], b_sb["k"][:, g:g + 1]))
                return ops

            def vproj_ops(c0, c1):
                ops = []
                for c in range(c0, c1):
                    j, o = c // 4, (c % 4) * 128
                    ps = ps_tile(name="vps")
                    ops.append(lambda ps=ps: nc.tensor.matmul(
                        ps[:, :C], lhsT=ones_sb, rhs=bvr_sb,
                        start=True, stop=False))
                    for dc in range(DC):
                        ops.append(lambda ps=ps, j=j, o=o, dc=dc:
                                   nc.tensor.matmul(
                                       ps[:, :C],
                                       lhsT=xvb[j][:, dc, o:o + 128],
                                       rhs=w_sb["v"][:, dc, :],
                                       start=False, stop=(dc == DC - 1)))
                    ops.append(lambda ps=ps, c=c: nc.vector.tensor_copy(
                        out=v_all[:, c, :, 0:DK],
                        in_=ps[:, :C].rearrange("p (h d) -> p h d", h=HPC)))
                return ops

            def qproj_ops(lb):
                ops = []
                for g in range(CT):
                    ps = ps_tile(name="qps")
                    ops.append(lambda ps=ps: prime(ps, w_rs["q"][:, :512]))
                    for dc in range(DC):
                        ops.append(lambda ps=ps, lb=lb, g=g, dc=dc:
                                   nc.tensor.matmul(
                                       ps,
                                       lhsT=w_sb["q"][:, dc, ts(g, 128)],
                                       rhs=xqb[lb][:, dc, :],
                                       start=False, stop=(dc == DC - 1)))
                    ops.append(lambda ps=ps, lb=lb, g=g:
                               nc.vector.tensor_scalar_add(
                                   QT[:, g, ts(lb, 512)], ps,
                                   b_sb["q"][:, g:g + 1]))
                return ops

            # softmax normalization of bank lb (runs as background ops in
            # bank lb+1): recip (DVE) -> ones-matmul bcast (PE) -> rep evac
            # (DVE) -> multiply (DVE, all-bf16 SBUF)
            def norm_ops(lb, otc):
                ops = []
                rcs, reps = [], []
                for h in range(HPC):
                    def op_rc(h=h):
                        rc = normp.tile([1, 512], MDT, tag="rc")
                        with nc.allow_low_precision(
                                reason="bf16 softmax denominator recip; "
                                       "|denom|>=1"):
                            nc.vector.reciprocal(rc, otc[h][DK:DK + 1, :])
                        rcs.append(rc)
                    ops.append(op_rc)
                for h in range(HPC):
                    def op_bc(h=h):
                        ps_rep = ps_tile(name="ps_rep")
                        nc.tensor.matmul(ps_rep[:DK, :],
                                         lhsT=ones_sb[:, :DK],
                                         rhs=rcs[h], start=True, stop=True)
                        reps.append(ps_rep)
                    ops.append(op_bc)
                for h in range(HPC):
                    def op_rep(h=h):
                        rep = normp.tile([64, 512], MDT, tag="rep")
                        nc.vector.tensor_copy(out=rep, in_=reps[h][:DK, :])
                        rcs[h] = rep  # reuse slot list to keep refs
                    ops.append(op_rep)
                for h in range(HPC):
                    def op_mul(h=h, lb=lb):
                        g, po = h // 2, 64 * (h % 2)
                        nc.vector.tensor_mul(
                            ot_sb[po:po + DK, g, ts(lb, 512)],
                            otc[h][0:DK, :], rcs[h])
                    ops.append(op_mul)
                return ops

            def oproj_ops(lb):
                ops = []
                for tt in range(4):
                    t = lb * 4 + tt
                    for half in range(2):
                        wps = ps_tile(name="wps")
                        for g in range(CT):
                            ops.append(lambda wps=wps, t=t, half=half, g=g:
                                       nc.tensor.matmul(
                                           wps,
                                           lhsT=ot_sb[:, g, ts(t, 128)],
                                           rhs=wo_sb[:, g, ts(half, 512)],
                                           start=(g == 0),
                                           stop=(g == CT - 1)))

                        def op_stage(wps=wps, t=t, half=half):
                            og = ostgp.tile([128, 512], MDT, tag="os",
                                            name="ostg")
                            nc.vector.tensor_copy(out=og, in_=wps)
                            od = nc.sync.dma_start(
                                out=out[ts(t, 128), ts(half, 512)], in_=og)
                            add_dep_helper(od.ins, last_dma[0].ins,
                                           sync=False, reason="odma-order")
                            last_dma[0] = od
                        ops.append(op_stage)
                return ops

            # ---- foreground prologue: K(j0), Q(0), V(c<VF) ----
            for op in kproj_ops(0):
                op()
            const_prime(w_sb["q"][:, 0, 0:1], w_sb["q"][:, 0:2, :])
            for op in qproj_ops(0):
                op()
            const_prime(w_sb["v"][:, 0, 0:1], w_sb["v"][:, 0:2, :])
            for op in vproj_ops(0, VF):
                op()
            const_prime(v_all[:, 0, 0, DK:DK + 1], v_all[:, :, :, DK])
            const_prime(wo_sb[:, 0, 0:1], wo_sb[:, 0, 0:512])

            # ---- per-Lq-bank software pipeline ----
            norm_pend = None   # norm ops of bank lb-1
            for lb in range(LB):
                bg = []
                if lb == 0:
                    for j in range(1, KB):
                        bg += kproj_ops(j)
                    bg += vproj_ops(VF, LTk)
                if norm_pend:
                    bg += norm_pend
                    bg += oproj_ops(lb - 1)
                if lb + 1 < LB:
                    bg += qproj_ops(lb + 1)

                # attention: 4 heads interleaved, scores kept transposed
                ot_ps = [ps_tile(name=f"ot_ps_{lb}_{h}") for h in range(HPC)]
                last_prime = None
                for h in range(HPC):
                    pp = prime(ot_ps[h][:DK + 1, :], QT[:, 0, ts(lb, 512)],
                               lhsT=zt[:, :DK + 1])
                    if last_prime is not None:
                        add_dep_helper(pp.ins, last_prime.ins, sync=False,
                                       reason="prime-order")
                    last_prime = pp
                otc = []
                for c in range(LTk):
                    for h in range(HPC):
                        g, po = h // 2, 64 * (h % 2)
                        s_ps = ps_tile(name="s_ps")
                        smi = nc.tensor.matmul(
                            s_ps,
                            lhsT=KT[po:po + DK, g, ts(c, 128)],
                            rhs=QT[po:po + DK, g, ts(lb, 512)],
                            start=True, stop=True)
                        add_dep_helper(smi.ins, last_prime.ins, sync=False,
                                       reason="s-after-prime")
                        p_t = work.tile([128, 512], MDT, tag="p")
                        nc.scalar.activation(
                            p_t, s_ps, mybir.ActivationFunctionType.Exp,
                            bias=mb_sb[:, c:c + 1], scale=1.0 / SCALE)
                        nc.tensor.matmul(
                            ot_ps[h][:DK + 1, :],
                            lhsT=v_all[:, c, h, :],
                            rhs=p_t,
                            start=False, stop=(c == LTk - 1))
                        if c == LTk - 1:
                            # evacuate + free the psum bank immediately
                            oc = work.tile([65, 512], MDT, tag="otc",
                                           name="otc")
                            nc.vector.tensor_copy(
                                out=oc, in_=ot_ps[h][:DK + 1, :])
                            otc.append(oc)
                    take = -(-len(bg) // (LTk - c))  # spread over c-iters
                    for op in bg[:take]:
                        op()
                    bg = bg[take:]

                norm_pend = norm_ops(lb, otc)

            # ---- epilogue: normalize + project the last bank ----
            for op in norm_pend:
                op()
            for op in oproj_ops(LB - 1):
                op()

    nc.compile()
    _strip_implied_dma_ring_waits(nc)
    return nc


def _strip_implied_dma_ring_waits(nc):
    """Drop DMA ring-semaphore waits implied by a compute-engine wait on the
    same descriptor (DMA descriptors carry a single hardware sync-wait).
    Applied to the x-block loads and output-staging stores, whose only
    DMA-semaphore deps are WAW/WAR-release edges already covered by the
    readers' engine semaphore."""
    import concourse.mybir as _mb
    for ins in nc.inst_map.values():
        if type(ins).__name__ != "InstDMACopy":
            continue
        if not ins.outs:
            continue
        memref = getattr(ins.outs[0], "memref", "") or ""
        src_ref = getattr(ins.ins[0], "memref", "") if ins.ins else ""
        if not (memref.startswith("xb") or (src_ref or "").startswith("ostg")):
            continue
        si = ins.sync_info
        if not si or not si.on_wait or len(si.on_wait) < 2:
            continue
        eng = [w_ for w_ in si.on_wait
               if not (w_.ant_name or "").startswith(("DMAHW", "DMASW"))]
        if not eng:
            continue
        ins.sync_info = _mb.SyncInfo(on_wait=eng, on_update=list(si.on_update))


def make_in_maps(query, key, value, mask, Wq, bq, Wk, bk, Wv, bv, Wo, bo,
                 LkP, mm_dtype=mybir.dt.bfloat16):
    """Host-side sharding + key compaction: per-core input dicts."""
    LTk = LkP // 128
    mdt = mybir.dt.np(mm_dtype)
    ones = np.ones((1, 128), np.float32).astype(mdt)
    ztc = np.zeros((128, 128), np.float32).astype(mdt)
    vones = np.ones((128, LTk * HPC), np.float32).astype(mdt)
    in_maps = []
    xTs, mbs = {}, {}
    for b in range(B):
        keep = np.flatnonzero(~mask[b, 0])
        n = len(keep)
        xkc = np.zeros((D_MODEL, LkP), mdt)
        xvc = np.zeros((D_MODEL, LkP), mdt)
        xkc[:, :n] = key[b].T[:, keep].astype(mdt)
        xvc[:, :n] = value[b].T[:, keep].astype(mdt)
        xTs[b] = {
            "q": np.ascontiguousarray(query[b].T.astype(mdt)),
            "k": xkc,
            "v": xvc,
        }
        mbf = np.full(LkP, np.float32(MASK_BIAS))
        mbf[:n] = 0.0
        mbs[b] = np.ascontiguousarray(mbf.reshape(LTk, 128).T)
    for c in range(N_CORES):
        b, hg = divmod(c, N_CORES // B)
        sl = slice(hg * C, (hg + 1) * C)
        in_maps.append({
            "xqT": xTs[b]["q"], "xkT": xTs[b]["k"], "xvT": xTs[b]["v"],
            "wq": np.ascontiguousarray(Wq[:, sl].astype(mdt)),
            "wk": np.ascontiguousarray(Wk[:, sl].astype(mdt)),
            "wv": np.ascontiguousarray(Wv[:, sl].astype(mdt)),
            "wo": np.ascontiguousarray(Wo[sl, :].astype(mdt)),
            "bq": np.ascontiguousarray(bq[sl].astype(np.float32)),
            "bk": np.ascontiguousarray(bk[sl].astype(np.float32)),
            "bvr": np.ascontiguousarray(bv[sl].astype(mdt)[None, :]),
            "mb": mbs[b],
            "ones": ones, "ztc": ztc, "vones": vones,
        })
    return in_maps


_NC_CACHE = {}


def _get_nc(LkP, mm_dtype=mybir.dt.bfloat16):
    key = (str(mm_dtype), LkP)
    if key not in _NC_CACHE:
        _NC_CACHE[key] = build_nc(LkP, mm_dtype=mm_dtype)
    return _NC_CACHE[key]


def run(inputs, mm_dtype=mybir.dt.bfloat16, trace=False):
    """Run on 8 cores; returns (full_output, BassKernelResults)."""
    inputs = {k: np.asarray(v) for k, v in inputs.items()}
    mask = inputs["mask"]
    counts = [int((~mask[b, 0]).sum()) for b in range(B)]
    LkP = max(128, 128 * int(math.ceil(max(counts) / 128.0)))
    nc = _get_nc(LkP, mm_dtype)
    in_maps = make_in_maps(**inputs, LkP=LkP, mm_dtype=mm_dtype)
    res = run_bass_kernel_spmd(nc, in_maps, list(range(N_CORES)), trace=trace)
    groups_per_batch = N_CORES // B
    out = np.zeros((B, L, D_MODEL), np.float32)
    for b in range(B):
        acc = np.zeros((L, D_MODEL), np.float32)
        if counts[b] > 0:
            for hg in range(groups_per_batch):
                acc += np.asarray(
                    res.results[b * groups_per_batch + hg]["partial"]
                ).astype(np.float32)
        out[b] = acc + inputs["bo"][None, :]
    return out, res


def kernel(**inputs) -> np.ndarray:
    out, _ = run(inputs)
    return out
